# revision 34
# baseline (speedup 1.0000x reference)
"""Trainium2 Bass kernel for nn_PixelTransformer (v2).

Math notes (valid for ANY input values; derived from reference semantics):
  * The transformer state is built purely from positional encodings
    (x never enters it); attention across batch with identical tokens
    reduces to v, so attention + residual folds to a per-layer 5x5 map.
  * Unscaled representation: carry u (= centered layer output, unscaled)
    with normalizer t = sqrt((1/5)|y|^2); the true post-LN state
    D(u/t) + e folds into the next layer's matrices.  LN eps terms are
    ~1e-5 relative (far below bf16 noise) and are dropped.
  * The LN1 input y1 = M u + cc*s is materialized in PSUM (bias added
    via fp8 DoubleRow once s = sqrt2 lands); the FFN mains fold M into
    w1 (G1 = W1u M) so they start straight from the u copy, overlapping
    the whole variance chain.
  * All rank-1 bias terms (B1 (x) t1 + (W1u cc) (x) s etc.) are fp8e4
    DoubleRow matmuls against an fp8 [t1|s] "std stack": two outer
    products per instruction at 0.5 cycles/column.  DoubleRow weight
    pair-strides must be multiples of 16, so 5/16-wide slots are padded
    and psy1/psy2 are 16-partition PSUM tiles (rows 5..15 unused).
  * mm2 runs as fp8 DoubleRow on chunk pairs (relu writes f in fp8),
    8 matmuls per layer; tok is centered by t0=[-1,0,1,0,1] on the host
    so it survives bf16.
  * PSUM-reading elementwise ops are legal only on DVE/ACT (the HW
    verifier rejects Pool/GPSIMD PSUM access); relus alternate DVE/ACT,
    Pool handles SBUF-only ops and half the input-DMA queue (the other
    half on SP), with per-layer streaming of w1/bias tensors.
  * ACT tables: one warm Sqrt load (sqrt set also covers Square/Relu)
    forced before layer 0, one swap to exp_and_others (Tanh+Exp) in the
    head, largely hidden behind the rec8/hid/psss chain.
  * The flow scan has the closed form z = exp(S) x + sum_j exp(...) t_j
    via one triangular matmul extended with a full-sum row at partition
    32 (PE rhs base must be 0/32/64); z^2 is reduced on-device to [32,1]
    and combined with the s-sums on the host (exp(sfac) is folded into
    the head weights host-side).

Sharding: N=1024 pixels split over 8 cores (128 each); weights replicated.
"""

import numpy as np

B, H, W = 32, 32, 32
N = H * W
L, D, FF = 8, 5, 2048
NCORES = 8
NP = N // NCORES
NCHUNK = FF // 128        # 16
NBANK = 4                 # psf banks of [128, 512]
EPS = 1e-5

_PROG = None


def _build_program():
    import concourse.bacc as bacc
    import concourse.mybir as mybir
    import concourse.tile as tile

    f32 = mybir.dt.float32
    bf16 = mybir.dt.bfloat16
    fp8 = mybir.dt.float8e4
    AF = mybir.ActivationFunctionType
    ALU = mybir.AluOpType
    AX = mybir.AxisListType
    PM = mybir.MatmulPerfMode

    nc = bacc.Bacc(name="pixel_transformer2")

    # crit: [5, 240] bf16: tokc 0:128 | M.T x8 128:168 | R.T x8 168:208 |
    #       crow x8 208:216 | H1.T 216:232 | row0: qc x8 at 232:240
    crit_d = nc.dram_tensor("crit", [5, 400], bf16, kind="ExternalInput")
    wm_d = [nc.dram_tensor(f"wm{l}", [5, FF], bf16, kind="ExternalInput")
            for l in range(L)]
    # per-layer bFF pair [1, 2*FF]; brest: biasR x8 | head x3 | bCC x8
    bff_d = [nc.dram_tensor(f"bff{l}", [1, 2 * FF], fp8, kind="ExternalInput")
             for l in range(L)]
    brest_d = nc.dram_tensor("brest", [1, 608], fp8, kind="ExternalInput")
    w2p_d = nc.dram_tensor("w2p", [128, 256 * L], fp8, kind="ExternalInput")
    headw_d = nc.dram_tensor("headw", [16, 65], bf16, kind="ExternalInput")
    xsh_d = nc.dram_tensor("xsh", [B, NP], f32, kind="ExternalInput")
    out_d = nc.dram_tensor("out48", [48, 1], f32, kind="ExternalOutput")

    with tile.TileContext(nc) as tc:
        with (
            tc.tile_pool(name="consts", bufs=1) as cp,
            tc.tile_pool(name="work", bufs=2) as wp,
            tc.tile_pool(name="fsb", bufs=4) as fp,
            tc.tile_pool(name="ps", bufs=2, space="PSUM") as pp,
        ):
            # ---- constants ----
            # ACT warm: load the sqrt table set before layer 0 needs it
            from bass_rust import add_dep_helper
            warmt = cp.tile([1, 1], f32)
            nc.vector.memset(warmt, 1.0)
            warmo = cp.tile([1, 1], f32)
            warm_inst = nc.scalar.activation(out=warmo, in_=warmt, func=AF.Sqrt)
            vconst = cp.tile([5, 1], bf16)
            nc.vector.memset(vconst, 1.0 / D)
            ones16c = cp.tile([1, 16], bf16)
            nc.vector.memset(ones16c, 1.0)
            ones16x32 = cp.tile([16, B], bf16)
            nc.vector.memset(ones16x32, 1.0)
            ones33 = cp.tile([33, B], bf16)
            nc.vector.memset(ones33, 1.0)
            # std stacks: [1, 2*NP] fp8 per layer boundary; slot0=t1(l),
            # slot1=s(l) (=t2 of l-1).  stack[L] slot0 is unused (head): 0.
            stacks = [cp.tile([1, 2 * NP], fp8, name=f"stk{l}") for l in range(L + 1)]
            for s_ in stacks:
                nc.vector.memset(s_, 1.0)


            # ---- input DMAs (all on SP queue; Pool stays DMA-free) ----
            crit = cp.tile([5, 400], bf16)
            nc.sync.dma_start(out=crit, in_=crit_d[:, :])
            brest = cp.tile([1, 608], fp8)
            nc.gpsimd.dma_start(out=brest, in_=brest_d[:, :])
            wm = cp.tile([5, L * FF], bf16)
            bias8 = cp.tile([1, 2 * FF * L], fp8)
            w2p = cp.tile([128, 256 * L], fp8)
            nc.gpsimd.dma_start(out=wm[:, 0:FF], in_=wm_d[0][:, :])
            nc.sync.dma_start(out=bias8[0:1, 0:2048], in_=bff_d[0][:, 0:2048])
            nc.gpsimd.dma_start(out=bias8[0:1, 2048:4096],
                                in_=bff_d[0][:, 2048:4096])
            nc.sync.dma_start(out=w2p, in_=w2p_d[:, :])
            for l in range(1, L):
                q = nc.sync if l % 2 == 0 else nc.gpsimd
                q.dma_start(out=wm[:, FF * l:FF * (l + 1)], in_=wm_d[l][:, :])
                q2 = nc.gpsimd if l % 2 == 0 else nc.sync
                q2.dma_start(out=bias8[0:1, 4096 * l:4096 * (l + 1)],
                             in_=bff_d[l][:, :])
            headw = cp.tile([16, 65], bf16)
            nc.sync.dma_start(out=headw, in_=headw_d[:, :])
            xsb = cp.tile([B, NP], f32)
            nc.gpsimd.dma_start(out=xsb, in_=xsh_d[:, :])

            tokc = crit[:, 0:128]
            MT = lambda l: crit[:, 128 + 16 * l:144 + 16 * l]
            RT = lambda l: crit[:, 256 + 16 * l:272 + 16 * l]
            H1T = crit[:, 384:400]

            def bFF(l, c):
                a = bias8[0:1, 4096 * l:4096 * (l + 1)].rearrange(
                    "p (two m) -> p two m", two=2)
                return a[:, :, 128 * c:128 * (c + 1)]

            def bR(l):
                return brest[0:1, 32 * l:32 * (l + 1)].rearrange(
                    "p (two m) -> p two m", two=2)

            def bHead(k):
                return brest[0:1, 256 + 32 * k:288 + 32 * k].rearrange(
                    "p (two m) -> p two m", two=2)

            def bCC(l):
                return brest[0:1, 352 + 32 * l:384 + 32 * l].rearrange(
                    "p (two m) -> p two m", two=2)

            def w2pair(l, p):
                return w2p[:, 256 * l + 32 * p:256 * l + 32 * (p + 1)].rearrange(
                    "p (two m) -> p two m", two=2)

            def stk(l):
                return stacks[l][0:1, 0:2 * NP].rearrange(
                    "p (two n) -> p two n", two=2)

            u_prev = tokc              # bf16 [5, NP]
            pending = None             # (sq2, l): boundary of prev layer

            def emit_boundary():
                """psv2 = vconst@sq2 (eps*t1^2 term is ~1e-5 relative --
                far below bf16 noise -- and dropped); sqrt -> stack slot1."""
                sq2p, lp = pending
                psv2 = pp.tile([1, NP], f32, tag="psmall", bufs=3,
                               name=f"psv2{lp}")
                nc.tensor.matmul(psv2, vconst, sq2p, start=True, stop=True)
                nc.scalar.activation(
                    out=stacks[lp + 1][0:1, NP:2 * NP], in_=psv2, func=AF.Sqrt)

            for l in range(L):
                # PE: y1 = M @ u (bias added after sqrt2) ; psy2: R @ u
                # pending boundary of layer l-1 -> psv2 / sqrt2 / stack slot1
                if pending is not None:
                    emit_boundary()

                psy1 = pp.tile([16, NP], f32, tag="psmall", bufs=3,
                               name=f"psy1{l}")
                ma_inst = nc.tensor.matmul(psy1, MT(l), u_prev,
                                           start=True, stop=False)
                if l == 0:
                    add_dep_helper(ma_inst.ins, warm_inst.ins,
                                   reason="sqrt act table warm before layer 0")
                psy2 = pp.tile([16, NP], f32, tag="pacc", bufs=1, name=f"psy2{l}")
                nc.tensor.matmul(psy2, RT(l), u_prev, start=True, stop=False)

                # PE: mains 0..8
                psfs = []
                for q in range(NBANK):
                    psf = pp.tile([128, 512], f32, tag="pf", bufs=4,
                                  name=f"psf{l}_{q}")
                    psfs.append(psf)
                for c in range(0, 2):
                    q, c4 = divmod(c, 4)
                    nc.tensor.matmul(
                        psfs[q][:, 128 * c4:128 * (c4 + 1)],
                        wm[:, FF * l + 128 * c:FF * l + 128 * (c + 1)],
                        u_prev, start=(c4 == 0), stop=False,
                    )

                # PE: y1 bias = cc (x) s (DoubleRow, slot0 zero), then close
                nc.tensor.matmul(psy1, bCC(l), stk(l), start=False, stop=True,
                                 perf_mode=PM.DoubleRow)

                # PE: mains 2..15
                for c in range(2, NCHUNK):
                    q, c4 = divmod(c, 4)
                    nc.tensor.matmul(
                        psfs[q][:, 128 * c4:128 * (c4 + 1)],
                        wm[:, FF * l + 128 * c:FF * l + 128 * (c + 1)],
                        u_prev, start=(c4 == 0), stop=False,
                    )

                # ACT: sq1 = y1^2 ; PE: psv1 = vconst@sq1 ; ACT: sqrt1
                sq1 = wp.tile([D, NP], bf16, tag="sq1", name=f"sq1{l}")
                nc.scalar.activation(out=sq1, in_=psy1[0:D, :], func=AF.Square)
                psv1 = pp.tile([1, NP], f32, tag="psmall", bufs=3,
                               name=f"psv1{l}")
                nc.tensor.matmul(psv1, vconst, sq1, start=True, stop=True)
                nc.scalar.activation(
                    out=stacks[l][0:1, 0:NP], in_=psv1, func=AF.Sqrt)

                # PE: biasFF DoubleRow, bank order 3,2,1,0
                chunk_order = [12, 13, 14, 15, 8, 9, 10, 11, 4, 5, 6, 7, 0, 1, 2, 3]
                for c in chunk_order:
                    q, c4 = divmod(c, 4)
                    nc.tensor.matmul(
                        psfs[q][:, 128 * c4:128 * (c4 + 1)],
                        bFF(l, c), stk(l), start=False, stop=(c4 == 3),
                        perf_mode=PM.DoubleRow,
                    )
                # PE: biasR DoubleRow into psy2
                nc.tensor.matmul(psy2, bR(l), stk(l), start=False, stop=False,
                                 perf_mode=PM.DoubleRow)

                # relu pieces -> f fp8 (PSUM readers: DVE/ACT only)
                fqs = []
                for q in range(NBANK):
                    fq = fp.tile([128, 512], fp8, tag="f", name=f"f{l}_{q}")
                    fqs.append(fq)
                nc.vector.tensor_scalar(out=fqs[3], in0=psfs[3], scalar1=0.0,
                                        scalar2=None, op0=ALU.max)
                nc.scalar.activation(out=fqs[2], in_=psfs[2], func=AF.Relu)
                nc.vector.tensor_scalar(out=fqs[1], in0=psfs[1], scalar1=0.0,
                                        scalar2=None, op0=ALU.max)
                nc.scalar.activation(out=fqs[0], in_=psfs[0], func=AF.Relu)

                # PE: mm2 DoubleRow pairs, in relu completion order
                for q in (3, 2, 1, 0):
                    for half in range(2):
                        last = (q == 0 and half == 1)
                        fpair = fqs[q][:, 256 * half:256 * (half + 1)].rearrange(
                            "p (two n) -> p two n", two=2)
                        nc.tensor.matmul(
                            psy2, w2pair(l, 2 * q + half), fpair,
                            start=False, stop=last, perf_mode=PM.DoubleRow,
                        )

                # ---- boundary: u2b copy + sq2 now; psv2/sqrt deferred ----
                sq2 = wp.tile([D, NP], bf16, tag="sq2", name=f"sq2{l}")
                nc.scalar.activation(out=sq2, in_=psy2[0:D, :], func=AF.Square)
                u2b = wp.tile([D, NP], bf16, tag="u2b", bufs=2, name=f"u2b{l}")
                nc.vector.tensor_copy(out=u2b, in_=psy2[0:D, :])

                u_prev = u2b
                pending = (sq2, l)

            # ================= head =================
            u8b, stack9 = u_prev, stacks[L]
            emit_boundary()                    # psv2_8 -> s8 = stack9 slot1
            # PE: psh = H1@u8 + hb1 (x) s8   [16, NP]
            psh = pp.tile([16, NP], f32, tag="psmall", bufs=3, name="psh")
            nc.tensor.matmul(psh, H1T, u8b, start=True, stop=False)
            nc.tensor.matmul(psh, bHead(0), stk(L), start=False, stop=True,
                             perf_mode=PM.DoubleRow)
            # DVE: rec8 = 1/s8 -> bf16
            rec8 = wp.tile([1, NP], bf16, tag="rec8")
            with nc.allow_low_precision(reason="1/s8 feeds bf16 matmul"):
                nc.vector.reciprocal(out=rec8, in_=stack9[0:1, NP:2 * NP])
            # Pool: hid = relu(psh) -> bf16
            hid = wp.tile([16, NP], bf16, tag="hid")
            nc.vector.tensor_scalar(out=hid, in0=psh, scalar1=0.0,
                                    scalar2=None, op0=ALU.max)
            # PE: psss/psst ; r8bc
            psss = pp.tile([16, NP], f32, tag="psmall", bufs=3, name="psss")
            nc.tensor.matmul(psss, headw[:, 33:49], hid, start=True, stop=False)
            nc.tensor.matmul(psss, bHead(1), stk(L), start=False, stop=True,
                             perf_mode=PM.DoubleRow)
            psst = pp.tile([16, NP], f32, tag="pacc", bufs=1, name="psst")
            nc.tensor.matmul(psst, headw[:, 49:65], hid, start=True, stop=False)
            nc.tensor.matmul(psst, bHead(2), stk(L), start=False, stop=True,
                             perf_mode=PM.DoubleRow)
            r8bc = pp.tile([16, NP], f32, tag="pf", bufs=4, name="r8bc")
            nc.tensor.matmul(r8bc, ones16c, rec8, start=True, stop=True)
            r8bcs = wp.tile([16, NP], bf16, tag="r8bcs")
            nc.vector.tensor_copy(out=r8bcs, in_=r8bc)

            # DVE: s_sb = psss * r8bc (f32; = s_/sf); t_sb = psst * r8bc
            s_sb = wp.tile([16, NP], f32, tag="s_sb")
            nc.vector.tensor_tensor(out=s_sb, in0=psss, in1=r8bcs, op=ALU.mult)
            t_sb = wp.tile([16, NP], f32, tag="t_sb")
            nc.vector.tensor_tensor(out=t_sb, in0=psst, in1=r8bcs, op=ALU.mult)
            # DVE: ssum = reduce(s_sb) [16,1]
            ssum = wp.tile([16, 1], f32, tag="ssum")
            nc.vector.tensor_reduce(out=ssum, in_=s_sb, op=ALU.add, axis=AX.X)
            nc.sync.dma_start(out=out_d[0:16, :], in_=ssum)

            # ACT: th = tanh(s_sb)  (exp table set loads here)
            th = wp.tile([16, NP], bf16, tag="th")
            nc.scalar.activation(out=th, in_=s_sb, func=AF.Tanh)
            # PE: psDext = TRIext @ th [33, NP] (row 32 = full sum)
            psD = pp.tile([33, NP], f32, tag="psmall", bufs=3, name="psD")
            nc.tensor.matmul(psD, headw[:, 0:33], th, start=True, stop=True)
            # ACT: wexpall = exp(psDext) -> bf16
            wexp = wp.tile([33, NP], bf16, tag="wexp")
            nc.scalar.activation(out=wexp, in_=psD, func=AF.Exp)
            # Pool: wt = wexp[0:16] * t_sb -> bf16
            wt = wp.tile([16, NP], bf16, tag="wt")
            nc.gpsimd.tensor_tensor(out=wt, in0=wexp[0:16, :], in1=t_sb,
                                    op=ALU.mult)
            # PE: pscb = ones16x32 @ wt ; pseb = ones1x32 @ eS (partition 32)
            pscb = pp.tile([B, NP], f32, tag="pacc", bufs=1, name="pscb")
            nc.tensor.matmul(pscb, ones16x32, wt, start=True, stop=True)
            pseb = pp.tile([B, NP], f32, tag="pf", bufs=4, name="pseb")
            nc.tensor.matmul(pseb, ones33[32:33, :], wexp[32:33, :],
                             start=True, stop=True)
            # Pool: zt = x*pseb ; z = zt + pscb ; zsq = z*z   (all f32)
            zt = wp.tile([B, NP], f32, tag="zt")
            nc.vector.tensor_tensor(out=zt, in0=xsb, in1=pseb, op=ALU.mult)
            z = wp.tile([B, NP], f32, tag="z")
            nc.vector.tensor_tensor(out=z, in0=zt, in1=pscb, op=ALU.add)
            zsq = wp.tile([B, NP], f32, tag="zsq")
            nc.gpsimd.tensor_tensor(out=zsq, in0=z, in1=z, op=ALU.mult)
            # DVE: zr = reduce(zsq) [32,1]
            zr = wp.tile([B, 1], f32, tag="zr")
            nc.vector.tensor_reduce(out=zr, in_=zsq, op=ALU.add, axis=AX.X)
            nc.sync.dma_start(out=out_d[16:48, :], in_=zr)

    nc.finalize()
    return nc


def _fold_inputs(inp):
    """Host-side weight folding (float64, cast at the end)."""
    import ml_dtypes

    f8 = ml_dtypes.float8_e4m3fn
    C = np.eye(D) - np.ones((D, D)) / D
    g = lambda k: np.asarray(inp[k], dtype=np.float64)
    wqkv, bqkv, wo, bo = g("wqkv"), g("bqkv"), g("wo"), g("bo")
    w1, b1, w2, b2 = g("w1"), g("b1"), g("w2"), g("b2")
    ln1w, ln1b, ln2w, ln2b = g("ln1w"), g("ln1b"), g("ln2w"), g("ln2b")

    t0 = np.array([-1.0, 0.0, 1.0, 0.0, 1.0])

    crit = np.zeros((5, 400), np.float64)
    wm = np.zeros((5, L * FF), np.float64)
    bias8 = np.zeros((1, 2 * FF * L), np.float64)
    brest = np.zeros((1, 608), np.float64)
    w2p = np.zeros((128, 256 * L), np.float64)

    for l in range(L):
        Dl = np.diag(ln2w[l - 1]) if l > 0 else np.eye(D)
        el = ln2b[l - 1] if l > 0 else np.zeros(D)
        wv = wqkv[l][2 * D:3 * D, :]
        bv = bqkv[l][2 * D:3 * D]
        A0 = np.eye(D) + wo[l] @ wv
        ca = wo[l] @ bv + bo[l]
        M = C @ A0 @ Dl
        cc = C @ (A0 @ el + ca)
        if l == 0:
            cc = cc + C @ (A0 @ t0)
        W1u = w1[l] * ln1w[l][None, :]          # [FF, 5]
        B1 = b1[l] + w1[l] @ ln1b[l]            # [FF]
        G1 = W1u @ M                            # [FF, 5]
        R = C @ np.diag(ln1w[l]) @ M

        crit[:, 128 + 16 * l:133 + 16 * l] = M.T
        crit[:, 256 + 16 * l:261 + 16 * l] = R.T

        wm[:, FF * l:FF * (l + 1)] = G1.T
        bias8[0, 4096 * l:4096 * l + 2048] = B1
        bias8[0, 4096 * l + 2048:4096 * (l + 1)] = W1u @ cc
        brest[0, 32 * l:32 * l + 5] = C @ (ln1b[l] + b2[l])
        brest[0, 32 * l + 16:32 * l + 21] = C @ np.diag(ln1w[l]) @ cc
        brest[0, 352 + 32 * l + 16:352 + 32 * l + 21] = cc

        W2c = C @ w2[l]                         # [5, FF]
        for p in range(8):
            for i in range(2):
                c = 2 * p + i
                base = 256 * l + 32 * p + 16 * i
                w2p[:, base:base + 5] = W2c[:, 128 * c:128 * (c + 1)].T

    # head folds
    f0w1, f0b1 = g("f0w1"), g("f0b1")
    f0w2, f0b2 = g("f0w2"), g("f0b2")
    D8 = np.diag(ln2w[L - 1])
    e8 = ln2b[L - 1]
    sf = float(np.exp(np.asarray(inp["sfac"], dtype=np.float64)[0]))
    H1 = f0w1 @ D8                              # [16, 5]
    hb1 = f0b1 + f0w1 @ e8
    crit[:, 384:400] = H1.T
    brest[0, 272:288] = hb1                     # psh pair slot1
    brest[0, 304:320] = f0b2[0:16] / sf         # psss pair slot1
    brest[0, 336:352] = f0b2[16:32]             # psst pair slot1

    headw = np.zeros((16, 65), np.float64)
    for j in range(16):
        headw[j + 1:16, j] = sf                 # TRIext cols 0..15
    headw[:, 32] = sf                           # full-sum col (partition 32)
    headw[:, 33:49] = (f0w2[0:16, :] / sf).T
    headw[:, 49:65] = f0w2[16:32, :].T

    # positional tokens (fp32 ops to match reference), centered by t0
    xs = (np.arange(W, dtype=np.float32) / np.float32(1e4)).astype(np.float32)
    ys = (np.arange(H, dtype=np.float32) / np.float32(1e4)).astype(np.float32)
    sinx = np.broadcast_to(np.sin(xs)[None, :], (H, W)).reshape(N)
    cosx = np.broadcast_to(np.cos(xs)[None, :], (H, W)).reshape(N)
    siny = np.broadcast_to(np.sin(ys)[:, None], (H, W)).reshape(N)
    cosy = np.broadcast_to(np.cos(ys)[:, None], (H, W)).reshape(N)
    tok = np.stack(
        [-np.ones(N, np.float32), sinx, cosx, siny, cosy], axis=0
    ).astype(np.float64)
    tokc = tok - t0[:, None]                    # [5, N] tiny values
    xflat = np.asarray(inp["x"], dtype=np.float32)[:, 0].reshape(B, N)

    out = {
        "crit": crit.astype(ml_dtypes.bfloat16),
        "brest": brest.astype(f8),
        "w2p": w2p.astype(f8),
        "headw": headw.astype(ml_dtypes.bfloat16),
        "tokc": tokc.astype(ml_dtypes.bfloat16),
        "xsh": xflat,
        "sf": sf,
    }
    for l in range(L):
        out[f"wm{l}"] = np.ascontiguousarray(
            wm[:, FF * l:FF * (l + 1)]).astype(ml_dtypes.bfloat16)
        out[f"bff{l}"] = np.ascontiguousarray(
            bias8[0:1, 4096 * l:4096 * (l + 1)]).astype(f8)
    return out


def get_program():
    global _PROG
    if _PROG is None:
        _PROG = _build_program()
    return _PROG


def make_in_maps(inputs):
    arrs = _fold_inputs(inputs)
    shared_keys = (["brest", "w2p", "headw"]
                   + [f"wm{l}" for l in range(L)]
                   + [f"bff{l}" for l in range(L)])
    shared = {k: arrs[k] for k in shared_keys}
    in_maps = []
    for core in range(NCORES):
        sl = slice(core * NP, (core + 1) * NP)
        m = dict(shared)
        crit = arrs["crit"].copy()
        crit[:, 0:128] = arrs["tokc"][:, sl]
        m["crit"] = np.ascontiguousarray(crit)
        m["xsh"] = np.ascontiguousarray(arrs["xsh"][:, sl])
        in_maps.append(m)
    return in_maps, arrs["sf"]


def combine_outputs(outs, sf):
    """per-core [48,1]: rows 0:16 = ssum (s_/sf), rows 16:48 = z^2 sums."""
    s_tot = 0.0
    q_tot = 0.0
    for o in outs:
        o = np.asarray(o, dtype=np.float64).reshape(48)
        s_tot += o[0:16].sum() * sf
        q_tot += o[16:48].sum()
    sldj = B * s_tot - 0.5 * q_tot - B * N * 0.5 * np.log(2.0 * np.pi)
    return np.array(-sldj, dtype=np.float32)


def kernel(**inputs):
    from concourse.bass_utils import run_bass_kernel_spmd

    nc = get_program()
    in_maps, sf = make_in_maps(inputs)
    res = run_bass_kernel_spmd(nc, in_maps, core_ids=list(range(NCORES)))
    return combine_outputs([r["out48"] for r in res.results], sf)
